# revision 1
# baseline (speedup 1.0000x reference)
"""Causal multi-head attention (B=1, S=4096, D=768, H=12, d_head=64) on 8
Trainium2 NeuronCores.

Sharding: tensor-parallel over heads. 12 heads are mapped onto 16 head-slots
(2 per core); the 4 leftover heads are duplicated onto two slots of the same
core with their W_out rows pre-scaled by 0.5, keeping the SPMD program
uniform across cores. Each core computes Q/K/V projections for its 2 head
slots, causal flash-attention (exp without max-subtraction; softmax
denominator obtained free via an appended ones-column on V), and a partial
row-parallel out-projection. The host sums the 8 partial outputs and adds
b_out (the all-reduce step of the row-parallel out projection).

All matmuls run in float32r with K=128/M=128 (zero-padded where the logical
dims are 64/65) — f32r only hits 1 cycle/row on full 128-wide operands.
"""

import sys

sys.path.insert(0, "/opt/trn_rl_repo")

import numpy as np

import concourse.bass as bass
import concourse.tile as tile
from concourse import bacc, mybir
from concourse.bass_utils import run_bass_kernel_spmd

S = 4096
D = 768
HD = 64
P = 128
KC = D // P  # 6 contraction chunks for the projections
QT_W = 512  # query-tile width (psum free dim)
NQT = S // QT_W  # 8 query tiles
NKB = S // P  # 32 key blocks
NEG = -1e30

F32 = mybir.dt.float32
F32R = mybir.dt.float32r
AF = mybir.ActivationFunctionType

SLOTS = [(0, 1), (2, 3), (4, 5), (6, 7), (8, 8), (9, 9), (10, 10), (11, 11)]
SCALES = [(1.0, 1.0)] * 4 + [(0.5, 0.5)] * 4

_CACHED_NC = None


def build_nc():
    nc = bacc.Bacc("TRN2", target_bir_lowering=False, debug=False, num_devices=8)

    x_d = nc.declare_dram_parameter("x", [S, D], F32, isOutput=False)
    wq_d = nc.declare_dram_parameter("wq", [D, P], F32, isOutput=False)
    wk_d = nc.declare_dram_parameter("wk", [D, P], F32, isOutput=False)
    wv_d = nc.declare_dram_parameter("wv", [D, P], F32, isOutput=False)
    wo_d = nc.declare_dram_parameter("wo", [P, D], F32, isOutput=False)
    mask_d = nc.declare_dram_parameter("mask", [P, P], F32, isOutput=False)
    ident_d = nc.declare_dram_parameter("ident", [P, P], F32, isOutput=False)
    out_d = nc.declare_dram_parameter("out", [S, D], F32, isOutput=True)

    with tile.TileContext(nc) as tc:
        with (
            tc.tile_pool(name="const", bufs=1) as const,
            tc.tile_pool(name="big", bufs=1) as big,
        ):
            # ---- constants ----
            mask_s = const.tile([P, P], F32)
            nc.sync.dma_start(mask_s[:], mask_d[:])
            ident = const.tile([P, P], F32)
            nc.sync.dma_start(ident[:], ident_d[:])
            ident_r = const.tile([P, P], F32R)
            nc.vector.tensor_copy(ident_r[:], ident[:])
            wpool = const  # warmup matmuls: get the PE HAM to 2.4 GHz while
            # the x DMA streams in
            ones_c = const.tile([P, 1], F32)
            nc.gpsimd.memset(ones_c[:], 1.0)
            zero_c = const.tile([P, 1], F32)
            nc.gpsimd.memset(zero_c[:], 0.0)
            wo_r = const.tile([P, D], F32R)

            # qT: slot A rows 0:64, slot B rows 64:128 (no padding needed on
            # the rhs side of the scores matmul). kT per slot, zero-padded on
            # the other 64 rows so the K=128 contraction only picks up its
            # slot. vA: V natural +ones column at 64, zero cols 65:128/slot.
            qT = big.tile([P, S], F32R)
            k2 = [big.tile([P, S], F32R, name=f"k2_{i}") for i in (0, 1)]
            vA = big.tile([P, NKB, 2 * P], F32R)

            nc.vector.tensor_copy(
                k2[0][64:P, :], zero_c[0:64, 0:1].broadcast_to([64, S])
            )
            nc.vector.tensor_copy(
                k2[1][0:64, :], zero_c[0:64, 0:1].broadcast_to([64, S])
            )
            for slot in (0, 1):
                nc.vector.tensor_copy(
                    vA[:, :, slot * P + 65 : slot * P + P],
                    zero_c[:, 0:1].broadcast_to([P, NKB, 63]),
                )
                nc.vector.tensor_copy(
                    vA[:, :, slot * P + 64],
                    ones_c[:, 0:1].broadcast_to([P, NKB]),
                )

            with (
                tc.tile_pool(name="xtp", bufs=1) as xtp,
                tc.tile_pool(name="psA", bufs=2, space="PSUM") as psA,
                tc.tile_pool(name="psB", bufs=4, space="PSUM") as psB,
            ):
                for wi in range(48):
                    wps = psA.tile([P, P], F32, name="tp", tag="tp")
                    nc.tensor.matmul(
                        wps[:], ident_r[:], ident_r[:], start=True, stop=True
                    )
                w_r = xtp.tile([P, KC, 3 * P], F32R)
                with tc.tile_pool(name="wst", bufs=1) as wst:
                    w_stage = wst.tile([P, KC, 3 * P], F32)
                    nc.sync.dma_start(
                        w_stage[:, :, 0:P], wq_d.rearrange("(c p) m -> p c m", p=P)
                    )
                    nc.sync.dma_start(
                        w_stage[:, :, P : 2 * P],
                        wk_d.rearrange("(c p) m -> p c m", p=P),
                    )
                    nc.sync.dma_start(
                        w_stage[:, :, 2 * P : 3 * P],
                        wv_d.rearrange("(c p) m -> p c m", p=P),
                    )
                    nc.vector.tensor_copy(w_r[:], w_stage[:])
                    wo_stage = wst.tile([P, D], F32)
                    nc.sync.dma_start(wo_stage[:], wo_d[:])
                    nc.vector.tensor_copy(wo_r[:], wo_stage[:])

                # ---- phases 1+2 interleaved: per q-tile group, DMA x,
                # transpose via PE, then Q/K/V projections for that group ----
                xT = xtp.tile([P, KC, S], F32R)
                with tc.tile_pool(name="xs", bufs=4) as xs:
                    for t in range(NQT):
                        for sti in range(4):
                            st = t * 4 + sti
                            for half in range(2):
                                x_stage = xs.tile([P, D // 2], F32)
                                nc.sync.dma_start(
                                    x_stage[:],
                                    x_d[
                                        st * P : (st + 1) * P,
                                        half * (D // 2) : (half + 1) * (D // 2),
                                    ],
                                )
                                for ci in range(KC // 2):
                                    c = half * (KC // 2) + ci
                                    tp = psA.tile([P, P], F32)
                                    nc.tensor.transpose(
                                        tp[:],
                                        x_stage[:, ci * P : (ci + 1) * P],
                                        ident[:],
                                    )
                                    nc.vector.tensor_copy(
                                        xT[:, c, st * P : (st + 1) * P], tp[:]
                                    )
                        # Q projection for this q-tile group
                        pj = psB.tile([P, QT_W], F32, name="pjq", tag="pj")
                        for c in range(KC):
                            nc.tensor.matmul(
                                pj[:],
                                w_r[:, c, 0:P],
                                xT[:, c, t * QT_W : (t + 1) * QT_W],
                                start=(c == 0),
                                stop=(c == KC - 1),
                            )
                        nc.vector.tensor_copy(qT[:, t * QT_W : (t + 1) * QT_W], pj[:])
                        # K projection
                        pj = psB.tile([P, QT_W], F32, name="pjk", tag="pj")
                        for c in range(KC):
                            nc.tensor.matmul(
                                pj[:],
                                w_r[:, c, P : 2 * P],
                                xT[:, c, t * QT_W : (t + 1) * QT_W],
                                start=(c == 0),
                                stop=(c == KC - 1),
                            )
                        nc.vector.tensor_copy(
                            k2[0][0:64, t * QT_W : (t + 1) * QT_W], pj[0:64, :]
                        )
                        nc.vector.tensor_copy(
                            k2[1][64:P, t * QT_W : (t + 1) * QT_W], pj[64:P, :]
                        )
                        # V projection + transpose to natural layout
                        pj = psB.tile([P, QT_W], F32, name="pjv", tag="pj")
                        for c in range(KC):
                            nc.tensor.matmul(
                                pj[:],
                                w_r[:, c, 2 * P : 3 * P],
                                xT[:, c, t * QT_W : (t + 1) * QT_W],
                                start=(c == 0),
                                stop=(c == KC - 1),
                            )
                        vt_t = xtp.tile(
                            [P, QT_W], F32R, name="vt_t", tag="vt_t", bufs=2
                        )
                        nc.vector.tensor_copy(vt_t[:], pj[:])
                        for b in range(QT_W // P):
                            kb = t * 4 + b
                            tp2 = psA.tile([P, P], F32R)
                            nc.tensor.transpose(
                                tp2[:], vt_t[:, b * P : (b + 1) * P], ident_r[:]
                            )
                            nc.vector.tensor_copy(vA[:, kb, 0:64], tp2[:, 0:64])
                            nc.vector.tensor_copy(
                                vA[:, kb, P : P + 64], tp2[:, 64:P]
                            )

            # ---- phase 3: attention ----
            cT = None
            with tc.tile_pool(name="ctx_sb", bufs=1) as ctx_sb:
              cT = ctx_sb.tile([P, S], F32R)
              with (
                tc.tile_pool(name="scp", bufs=4, space="PSUM") as scp,
                tc.tile_pool(name="ctp", bufs=2, space="PSUM") as ctp,
                tc.tile_pool(name="pt", bufs=8) as pt,
                tc.tile_pool(name="sm", bufs=4) as sm,
              ):
                def outproj(st):
                    o_stage = sm.tile([P, D], F32, name="o_stage", bufs=3)
                    for nch in range(2):
                        po = scp.tile([P, QT_W], F32, name="sc", tag="sc")
                        nc.tensor.matmul(
                            po[:, : D // 2],
                            cT[:, st * P : (st + 1) * P],
                            wo_r[:, nch * (D // 2) : (nch + 1) * (D // 2)],
                            start=True,
                            stop=True,
                        )
                        nc.vector.tensor_copy(
                            o_stage[:, nch * (D // 2) : (nch + 1) * (D // 2)],
                            po[:, : D // 2],
                        )
                    nc.sync.dma_start(out_d[st * P : (st + 1) * P, :], o_stage[:])

                for t in range(NQT):
                    if t == NQT - 1:
                        # rows covered by tiles 0-6 are final; overlap their
                        # out-projection with the last (largest) q-tile
                        for st in range(28):
                            outproj(st)
                    nkb = 4 * (t + 1)
                    ctx_ps = [
                        ctp.tile([P, QT_W], F32, name=f"ctx{s}", tag=f"ctx{s}")
                        for s in (0, 1)
                    ]
                    for kb in range(nkb):
                        r = kb * P - t * QT_W  # diagonal offset
                        r0 = max(0, r)
                        p_tiles = []
                        for slot in (0, 1):
                            sc = scp.tile([P, QT_W], F32, name="sc", tag="sc")
                            nc.tensor.matmul(
                                sc[:],
                                k2[slot][:, kb * P : (kb + 1) * P],
                                qT[:, t * QT_W : (t + 1) * QT_W],
                                start=True,
                                stop=True,
                            )
                            if r >= 0:
                                nc.vector.tensor_tensor(
                                    sc[:, r : r + P],
                                    sc[:, r : r + P],
                                    mask_s[:],
                                    mybir.AluOpType.add,
                                )
                            p_t = pt.tile([P, QT_W], F32R, name="ptile")
                            nc.scalar.activation(
                                p_t[:, r0:QT_W],
                                sc[:, r0:QT_W],
                                AF.Exp,
                                scale=0.125,
                            )
                            p_tiles.append(p_t)
                        for slot in (0, 1):
                            nc.tensor.matmul(
                                ctx_ps[slot][:, r0:QT_W],
                                vA[:, kb, slot * P : (slot + 1) * P],
                                p_tiles[slot][:, r0:QT_W],
                                start=(kb == 0),
                                stop=(kb == nkb - 1),
                            )
                    for slot in (0, 1):
                        lr = sm.tile([1, QT_W], F32, name="lrecip")
                        nc.vector.reciprocal(lr[:], ctx_ps[slot][64:65, :])
                        lb = sm.tile([64, QT_W], F32, name="lb")
                        nc.gpsimd.partition_broadcast(lb[:], lr[0:1, :])
                        nc.vector.tensor_tensor(
                            cT[slot * 64 : slot * 64 + 64, t * QT_W : (t + 1) * QT_W],
                            ctx_ps[slot][0:64, :],
                            lb[:],
                            mybir.AluOpType.mult,
                        )

                for st in range(28, S // P):
                    outproj(st)



    nc.compile()
    return nc


def _host_inputs(x, W_query, W_key, W_value, W_out):
    mask = np.where(
        np.arange(P)[:, None] <= np.arange(P)[None, :], 0.0, NEG
    ).astype(np.float32)
    ident = np.eye(P, dtype=np.float32)
    in_maps = []
    for core in range(8):
        ha, hb = SLOTS[core]
        sa, sb = SCALES[core]
        ca, cb = slice(ha * HD, (ha + 1) * HD), slice(hb * HD, (hb + 1) * HD)
        in_maps.append(
            {
                "x": np.ascontiguousarray(x),
                "wq": np.ascontiguousarray(
                    np.concatenate([W_query[:, ca], W_query[:, cb]], axis=1)
                ),
                "wk": np.ascontiguousarray(
                    np.concatenate([W_key[:, ca], W_key[:, cb]], axis=1)
                ),
                "wv": np.ascontiguousarray(
                    np.concatenate([W_value[:, ca], W_value[:, cb]], axis=1)
                ),
                "wo": np.ascontiguousarray(
                    np.concatenate([W_out[ca, :] * sa, W_out[cb, :] * sb], axis=0)
                ),
                "mask": mask,
                "ident": ident,
            }
        )
    return in_maps


def run(x, W_query, W_key, W_value, W_out, b_out, trace=False):
    global _CACHED_NC
    if _CACHED_NC is None:
        _CACHED_NC = build_nc()
    nc = _CACHED_NC
    in_maps = _host_inputs(x, W_query, W_key, W_value, W_out)
    res = run_bass_kernel_spmd(nc, in_maps, core_ids=list(range(8)), trace=trace)
    out = np.zeros((S, D), dtype=np.float32)
    for core in range(8):
        out += res.results[core]["out"]
    out += b_out[None, :].astype(np.float32)
    return out, res


def kernel(x, W_query, W_key, W_value, W_out, b_out):
    x2 = np.asarray(x, dtype=np.float32).reshape(S, D)
    out, _ = run(
        x2,
        np.asarray(W_query, np.float32),
        np.asarray(W_key, np.float32),
        np.asarray(W_value, np.float32),
        np.asarray(W_out, np.float32),
        np.asarray(b_out, np.float32),
    )
    return out.reshape(1, S, D)



# revision 10
# speedup vs baseline: 1.4380x; 1.4380x over previous
"""Causal multi-head attention (B=1, S=4096, D=768, H=12, d_head=64) on 8
Trainium2 NeuronCores.

Sharding: tensor-parallel over heads. 12 heads are mapped onto 16 head-slots
(2 per core); the 4 leftover heads are duplicated onto two slots of the same
core with their W_out rows pre-scaled by 0.5, keeping the SPMD program
uniform across cores. Each core computes Q/K/V projections for its 2 head
slots, causal flash-attention (exp without max-subtraction; softmax
denominator obtained free via an appended ones-column on V), and a partial
row-parallel out-projection. The host sums the 8 partial outputs and adds
b_out (the all-reduce step of the row-parallel out projection).

v2: bf16 data path. The host pre-casts x and all weights to bf16; x is
transposed on the fly by the DMA XBAR (dma transpose), so the PE runs no
transposes at all. Matmuls stream bf16 (1 cycle/row, lower power than
float32r -> less HAM throttling). Scores for the two head slots go to one
2-bank PSUM tile so a single activation instruction exponentiates both
slots. Softmax reciprocal uses the fast approximate DVE op. Per-q-tile
interleaving: project tile t, then attention for tile t, with tile t-1's
out-projection slotted between to hide the softmax-normalize latency.
"""

import sys

sys.path.insert(0, "/opt/trn_rl_repo")

import ml_dtypes
import numpy as np

import concourse.bass as bass
import concourse.tile as tile
from concourse import bacc, mybir
from concourse.bass_utils import run_bass_kernel_spmd

S = 4096
D = 768
HD = 64
P = 128
KC = D // P  # 6 contraction chunks for the projections
QT_W = 512  # query-tile width (psum free dim)
NQT = S // QT_W  # 8 query tiles
NKB = S // P  # 32 key blocks
NEG = -1e30

F32 = mybir.dt.float32
BF16 = mybir.dt.bfloat16
AF = mybir.ActivationFunctionType
BF = ml_dtypes.bfloat16

SLOTS = [(0, 1), (2, 3), (4, 5), (6, 7), (8, 8), (9, 9), (10, 10), (11, 11)]
SCALES = [(1.0, 1.0)] * 4 + [(0.5, 0.5)] * 4

_CACHED_NC = None


def build_nc():
    nc = bacc.Bacc("TRN2", target_bir_lowering=False, debug=False, num_devices=8)

    xt_d = nc.declare_dram_parameter("xt", [P, KC, S], BF16, isOutput=False)
    w_d = nc.declare_dram_parameter("w", [P, KC, 3 * P], BF16, isOutput=False)
    wo_d = nc.declare_dram_parameter("wo", [P, D], BF16, isOutput=False)
    mask_d = nc.declare_dram_parameter("mask", [P, P], F32, isOutput=False)
    out_d = nc.declare_dram_parameter("out", [S, D], F32, isOutput=True)

    with tile.TileContext(nc) as tc:
        with (
            tc.tile_pool(name="const", bufs=1) as const,
            tc.tile_pool(name="big", bufs=1) as big,
        ):
            # ---- constants / staging ----
            warm = const.tile([P, 256], BF16)
            nc.gpsimd.memset(warm[:], 0.0)
            mask_s = const.tile([P, P], F32)
            nc.sync.dma_start(mask_s[:], mask_d[:])
            w = const.tile([P, KC, 3 * P], BF16)
            nc.sync.dma_start(w[:], w_d[:])
            wo = const.tile([P, D], BF16)
            nc.sync.dma_start(wo[:], wo_d[:])

            # qT/kT: packed 2-slot layout straight from the projection PSUM:
            # slot s occupies partitions s*64:(s+1)*64; scores contract K=64.
            xT = big.tile([P, KC, S], BF16)
            qT = big.tile([P, S], BF16)
            kT = big.tile([P, S], BF16)
            # vA: per key block, per slot: [64 v-dims + ones column] so the PV
            # matmul's lhsT [128 keys, 65] yields ctx rows 0:64 and the
            # softmax denominator in row 64.
            vA = big.tile([P, NKB, 2, 65], BF16)
            nc.gpsimd.memset(vA[:, :, :, 64:65], 1.0)
            cT = big.tile([P, S], BF16)
            scratch = const.tile([P, P], F32)

            # xT comes pre-transposed from the host; stream it per (tile,
            # chunk) so projections only wait on their own slices.
            for t in range(NQT):
                for c in range(KC):
                    nc.sync.dma_start(
                        xT[:, c, t * QT_W : (t + 1) * QT_W],
                        xt_d[:, c, t * QT_W : (t + 1) * QT_W],
                    )

            with (
                tc.tile_pool(name="pjp", bufs=2, space="PSUM") as pjp,
                tc.tile_pool(name="scp", bufs=2, space="PSUM") as scp,
                tc.tile_pool(name="ctp", bufs=1, space="PSUM") as ctp,
                tc.tile_pool(name="vt", bufs=2) as vtp,
                tc.tile_pool(name="pt", bufs=4) as pt,
                tc.tile_pool(name="sm", bufs=4) as sm,
            ):
                # warm up the PE HAM + preload the Exp table while DMAs run
                for wi in range(48):
                    wps = pjp.tile([P, 256], F32, name="warm_ps", tag="pj")
                    nc.tensor.matmul(
                        wps[:], warm[:, 0:P], warm[:], start=True, stop=True
                    )
                nc.scalar.activation(scratch[:], mask_s[:], AF.Exp, scale=0.125)

                def outproj(t):
                    for st in range(4 * t, 4 * t + 4):
                        o_stage = sm.tile([P, D], F32, name="o_stage", bufs=3)
                        for nch in range(2):
                            po = pjp.tile([P, QT_W], F32, name="po", tag="pj")
                            nc.tensor.matmul(
                                po[:, : D // 2],
                                cT[:, st * P : (st + 1) * P],
                                wo[:, nch * (D // 2) : (nch + 1) * (D // 2)],
                                start=True,
                                stop=True,
                            )
                            nc.vector.tensor_copy(
                                o_stage[:, nch * (D // 2) : (nch + 1) * (D // 2)],
                                po[:, : D // 2],
                            )
                        nc.sync.dma_start(
                            out_d[st * P : (st + 1) * P, :], o_stage[:]
                        )

                for t in range(NQT):
                    tsl = slice(t * QT_W, (t + 1) * QT_W)
                    # ---- projections for q-tile t ----
                    for pi, dst in ((0, qT), (1, kT)):
                        pj = pjp.tile([P, QT_W], F32, name="pj", tag="pj")
                        for c in range(KC):
                            nc.tensor.matmul(
                                pj[:],
                                w[:, c, pi * P : (pi + 1) * P],
                                xT[:, c, tsl],
                                start=(c == 0),
                                stop=(c == KC - 1),
                            )
                        nc.vector.tensor_copy(dst[:, tsl], pj[:])
                    pj = pjp.tile([P, QT_W], F32, name="pj", tag="pj")
                    for c in range(KC):
                        nc.tensor.matmul(
                            pj[:],
                            w[:, c, 2 * P : 3 * P],
                            xT[:, c, tsl],
                            start=(c == 0),
                            stop=(c == KC - 1),
                        )
                    vt = vtp.tile([P, QT_W], BF16, name="vt")
                    nc.vector.tensor_copy(vt[:], pj[:])
                    # V back to natural [keys, vdims] layout via DMA XBAR.
                    # The XBAR needs a contiguous destination, so land in a
                    # staging tile and split the two slots with DVE copies.
                    for b in range(4):
                        kb = 4 * t + b
                        vN = vtp.tile([P, P], BF16, name="vN", bufs=3)
                        nc.sync.dma_start(
                            vN[:],
                            vt[:, b * P : (b + 1) * P],
                            transpose=True,
                        )
                        for slot in (0, 1):
                            nc.vector.tensor_copy(
                                vA[:, kb, slot, 0:64],
                                vN[:, slot * HD : (slot + 1) * HD],
                            )

                    # out-projection of the previous tile here: its PE work is
                    # independent, hiding the normalize latency of tile t-1
                    # while attention t's first matmuls wait on nothing.
                    if t > 0:
                        outproj(t - 1)

                    # ---- attention for q-tile t ----
                    nkb = 4 * (t + 1)
                    ctx = [
                        ctp.tile([P, QT_W], F32, name=f"ctx{s}", tag=f"ctx{s}")
                        for s in (0, 1)
                    ]
                    for kb in range(nkb):
                        r = kb * P - t * QT_W  # diagonal offset
                        r0 = max(0, r)
                        sc2 = scp.tile([P, 2, QT_W], F32, name="sc", tag="sc")
                        for slot in (0, 1):
                            ssl = slice(slot * HD, (slot + 1) * HD)
                            nc.tensor.matmul(
                                sc2[:, slot, r0:QT_W],
                                kT[ssl, kb * P : (kb + 1) * P],
                                qT[ssl, t * QT_W + r0 : (t + 1) * QT_W],
                                start=True,
                                stop=True,
                            )
                        if r >= 0:
                            nc.vector.tensor_tensor(
                                sc2[:, :, r : r + P],
                                sc2[:, :, r : r + P],
                                mask_s[:, None, :].broadcast_to([P, 2, P]),
                                mybir.AluOpType.add,
                            )
                        p2 = pt.tile([P, 2, QT_W], BF16, name="p2")
                        nc.scalar.activation(
                            p2[:, :, r0:QT_W],
                            sc2[:, :, r0:QT_W],
                            AF.Exp,
                            scale=0.125,
                        )
                        for slot in (0, 1):
                            nc.tensor.matmul(
                                ctx[slot][0:65, r0:QT_W],
                                vA[:, kb, slot, :],
                                p2[:, slot, r0:QT_W],
                                start=(kb == 0),
                                stop=(kb == nkb - 1),
                            )
                    # softmax normalization; cT rows 0:64 slot0, 64:128 slot1
                    for slot in (0, 1):
                        # the custom-DVE reciprocal can't read PSUM; stage the
                        # denominator row through SBUF first
                        dsb = sm.tile([1, QT_W], F32, name="dsb")
                        nc.vector.tensor_copy(dsb[:], ctx[slot][64:65, :])
                        lr = sm.tile([1, QT_W], F32, name="lr")
                        nc.vector.reciprocal_approx_fast(lr[:], dsb[:])
                        lb = sm.tile([64, QT_W], F32, name="lb")
                        nc.gpsimd.partition_broadcast(lb[:], lr[0:1, :])
                        nc.vector.tensor_tensor(
                            cT[slot * HD : (slot + 1) * HD, tsl],
                            ctx[slot][0:64, :],
                            lb[:],
                            mybir.AluOpType.mult,
                        )
                outproj(NQT - 1)

    nc.compile()
    return nc


def _host_inputs(x, W_query, W_key, W_value, W_out):
    mask = np.where(
        np.arange(P)[:, None] <= np.arange(P)[None, :], 0.0, NEG
    ).astype(np.float32)
    # host-side transpose: xt[p, c, s] = x[s, c*128 + p]
    xt = np.ascontiguousarray(
        x.astype(BF).T.reshape(KC, P, S).transpose(1, 0, 2)
    )
    in_maps = []
    for core in range(8):
        ha, hb = SLOTS[core]
        sa, sb = SCALES[core]
        ca, cb = slice(ha * HD, (ha + 1) * HD), slice(hb * HD, (hb + 1) * HD)
        # packed per-core projection weights [768, 128] -> [128(p), 6(c), 128]
        def pack(wm):
            sel = np.concatenate([wm[:, ca], wm[:, cb]], axis=1)  # [768, 128]
            return sel.reshape(KC, P, P).transpose(1, 0, 2)  # [p, c, m]

        wq, wk, wv = pack(W_query), pack(W_key), pack(W_value)
        w_all = np.concatenate([wq, wk, wv], axis=2).astype(BF)  # [128, 6, 384]
        wo = np.concatenate([W_out[ca, :] * sa, W_out[cb, :] * sb], axis=0).astype(
            BF
        )
        in_maps.append(
            {
                "xt": xt,
                "w": np.ascontiguousarray(w_all),
                "wo": np.ascontiguousarray(wo),
                "mask": mask,
            }
        )
    return in_maps


def run(x, W_query, W_key, W_value, W_out, b_out, trace=False):
    global _CACHED_NC
    if _CACHED_NC is None:
        _CACHED_NC = build_nc()
    nc = _CACHED_NC
    in_maps = _host_inputs(x, W_query, W_key, W_value, W_out)
    res = run_bass_kernel_spmd(nc, in_maps, core_ids=list(range(8)), trace=trace)
    out = np.zeros((S, D), dtype=np.float32)
    for core in range(8):
        out += res.results[core]["out"]
    out += b_out[None, :].astype(np.float32)
    return out, res


def kernel(x, W_query, W_key, W_value, W_out, b_out):
    x2 = np.asarray(x, dtype=np.float32).reshape(S, D)
    out, _ = run(
        x2,
        np.asarray(W_query, np.float32),
        np.asarray(W_key, np.float32),
        np.asarray(W_value, np.float32),
        np.asarray(W_out, np.float32),
        np.asarray(b_out, np.float32),
    )
    return out.reshape(1, S, D)


# revision 11
# speedup vs baseline: 1.4817x; 1.0304x over previous
"""Causal multi-head attention (B=1, S=4096, D=768, H=12, d_head=64) on 8
Trainium2 NeuronCores.

Sharding: tensor-parallel over heads. 12 heads are mapped onto 16 head-slots
(2 per core); the 4 leftover heads are duplicated onto two slots of the same
core with their W_out rows pre-scaled by 0.5, keeping the SPMD program
uniform across cores. Each core computes Q/K/V projections for its 2 head
slots, causal flash-attention (exp without max-subtraction; softmax
denominator obtained free via an appended ones-column on V), and a partial
row-parallel out-projection. The host sums the 8 partial outputs and adds
b_out (the all-reduce step of the row-parallel out projection).

v2: bf16 data path. The host pre-casts x and all weights to bf16; x is
transposed on the fly by the DMA XBAR (dma transpose), so the PE runs no
transposes at all. Matmuls stream bf16 (1 cycle/row, lower power than
float32r -> less HAM throttling). Scores for the two head slots go to one
2-bank PSUM tile so a single activation instruction exponentiates both
slots. Softmax reciprocal uses the fast approximate DVE op. Per-q-tile
interleaving: project tile t, then attention for tile t, with tile t-1's
out-projection slotted between to hide the softmax-normalize latency.
"""

import sys

sys.path.insert(0, "/opt/trn_rl_repo")

import ml_dtypes
import numpy as np

import concourse.bass as bass
import concourse.tile as tile
from concourse import bacc, mybir
from concourse.bass_utils import run_bass_kernel_spmd

S = 4096
D = 768
HD = 64
P = 128
KC = D // P  # 6 contraction chunks for the projections
QT_W = 512  # query-tile width (psum free dim)
NQT = S // QT_W  # 8 query tiles
NKB = S // P  # 32 key blocks
NEG = -1e30

F32 = mybir.dt.float32
BF16 = mybir.dt.bfloat16
AF = mybir.ActivationFunctionType
BF = ml_dtypes.bfloat16

SLOTS = [(0, 1), (2, 3), (4, 5), (6, 7), (8, 8), (9, 9), (10, 10), (11, 11)]
SCALES = [(1.0, 1.0)] * 4 + [(0.5, 0.5)] * 4

_CACHED_NC = None


def build_nc():
    nc = bacc.Bacc("TRN2", target_bir_lowering=False, debug=False, num_devices=8)

    xt_d = nc.declare_dram_parameter("xt", [P, KC, S], BF16, isOutput=False)
    w_d = nc.declare_dram_parameter("w", [P, KC, 3 * P], BF16, isOutput=False)
    wo_d = nc.declare_dram_parameter("wo", [P, D], BF16, isOutput=False)
    mask_d = nc.declare_dram_parameter("mask", [P, P], F32, isOutput=False)
    out_d = nc.declare_dram_parameter("out", [S, D], F32, isOutput=True)

    with tile.TileContext(nc) as tc:
        with (
            tc.tile_pool(name="const", bufs=1) as const,
            tc.tile_pool(name="big", bufs=1) as big,
        ):
            # ---- constants / staging ----
            warm = const.tile([P, 256], BF16)
            nc.gpsimd.memset(warm[:], 0.0)
            mask_s = const.tile([P, P], F32)
            nc.sync.dma_start(mask_s[:], mask_d[:])
            w = const.tile([P, KC, 3 * P], BF16)
            nc.sync.dma_start(w[:], w_d[:])
            wo = const.tile([P, D], BF16)
            nc.sync.dma_start(wo[:], wo_d[:])

            # qT/kT: packed 2-slot layout straight from the projection PSUM:
            # slot s occupies partitions s*64:(s+1)*64; scores contract K=64.
            xT = big.tile([P, KC, S], BF16)
            qT = big.tile([P, S], BF16)
            kT = big.tile([P, S], BF16)
            # vA: per key block, per slot: [64 v-dims + ones column] so the PV
            # matmul's lhsT [128 keys, 65] yields ctx rows 0:64 and the
            # softmax denominator in row 64.
            vA = big.tile([P, NKB, 2, 65], BF16)
            nc.gpsimd.memset(vA[:, :, :, 64:65], 1.0)
            cT = big.tile([P, S], BF16)
            scratch = const.tile([P, P], F32)

            # xT comes pre-transposed from the host; one DMA per q-tile (all 6
            # chunks) keeps the sync queue's per-DMA issue overhead off the
            # critical path while projections still only wait on their tile.
            for t in range(NQT):
                nc.sync.dma_start(
                    xT[:, :, t * QT_W : (t + 1) * QT_W],
                    xt_d[:, :, t * QT_W : (t + 1) * QT_W],
                )

            with (
                tc.tile_pool(name="pjp", bufs=2, space="PSUM") as pjp,
                tc.tile_pool(name="scp", bufs=2, space="PSUM") as scp,
                tc.tile_pool(name="ctp", bufs=1, space="PSUM") as ctp,
                tc.tile_pool(name="vt", bufs=2) as vtp,
                tc.tile_pool(name="pt", bufs=4) as pt,
                tc.tile_pool(name="sm", bufs=4) as sm,
            ):
                # warm up the PE HAM + preload the Exp table while DMAs run
                for wi in range(48):
                    wps = pjp.tile([P, 256], F32, name="warm_ps", tag="pj")
                    nc.tensor.matmul(
                        wps[:], warm[:, 0:P], warm[:], start=True, stop=True
                    )
                nc.scalar.activation(scratch[:], mask_s[:], AF.Exp, scale=0.125)

                def outproj(t):
                    for st in range(4 * t, 4 * t + 4):
                        o_stage = sm.tile([P, D], F32, name="o_stage", bufs=3)
                        for nch in range(2):
                            po = pjp.tile([P, QT_W], F32, name="po", tag="pj")
                            nc.tensor.matmul(
                                po[:, : D // 2],
                                cT[:, st * P : (st + 1) * P],
                                wo[:, nch * (D // 2) : (nch + 1) * (D // 2)],
                                start=True,
                                stop=True,
                            )
                            nc.vector.tensor_copy(
                                o_stage[:, nch * (D // 2) : (nch + 1) * (D // 2)],
                                po[:, : D // 2],
                            )
                        nc.sync.dma_start(
                            out_d[st * P : (st + 1) * P, :], o_stage[:]
                        )

                for t in range(NQT):
                    tsl = slice(t * QT_W, (t + 1) * QT_W)
                    # ---- projections for q-tile t ----
                    for pi, dst in ((0, qT), (1, kT)):
                        pj = pjp.tile([P, QT_W], F32, name="pj", tag="pj")
                        for c in range(KC):
                            nc.tensor.matmul(
                                pj[:],
                                w[:, c, pi * P : (pi + 1) * P],
                                xT[:, c, tsl],
                                start=(c == 0),
                                stop=(c == KC - 1),
                            )
                        nc.vector.tensor_copy(dst[:, tsl], pj[:])
                    pj = pjp.tile([P, QT_W], F32, name="pj", tag="pj")
                    for c in range(KC):
                        nc.tensor.matmul(
                            pj[:],
                            w[:, c, 2 * P : 3 * P],
                            xT[:, c, tsl],
                            start=(c == 0),
                            stop=(c == KC - 1),
                        )
                    vt = vtp.tile([P, QT_W], BF16, name="vt")
                    nc.vector.tensor_copy(vt[:], pj[:])
                    # V back to natural [keys, vdims] layout via DMA XBAR.
                    # The XBAR needs a contiguous destination, so land in a
                    # staging tile and split the two slots with DVE copies.
                    for b in range(4):
                        kb = 4 * t + b
                        vN = vtp.tile([P, P], BF16, name="vN", bufs=3)
                        nc.sync.dma_start(
                            vN[:],
                            vt[:, b * P : (b + 1) * P],
                            transpose=True,
                        )
                        for slot in (0, 1):
                            nc.vector.tensor_copy(
                                vA[:, kb, slot, 0:64],
                                vN[:, slot * HD : (slot + 1) * HD],
                            )

                    # out-projection of the previous tile here: its PE work is
                    # independent, hiding the normalize latency of tile t-1
                    # while attention t's first matmuls wait on nothing.
                    if t > 0:
                        outproj(t - 1)

                    # ---- attention for q-tile t ----
                    nkb = 4 * (t + 1)
                    ctx = [
                        ctp.tile([P, QT_W], F32, name=f"ctx{s}", tag=f"ctx{s}")
                        for s in (0, 1)
                    ]
                    for kb in range(nkb):
                        r = kb * P - t * QT_W  # diagonal offset
                        r0 = max(0, r)
                        sc2 = scp.tile([P, 2, QT_W], F32, name="sc", tag="sc")
                        for slot in (0, 1):
                            ssl = slice(slot * HD, (slot + 1) * HD)
                            nc.tensor.matmul(
                                sc2[:, slot, r0:QT_W],
                                kT[ssl, kb * P : (kb + 1) * P],
                                qT[ssl, t * QT_W + r0 : (t + 1) * QT_W],
                                start=True,
                                stop=True,
                            )
                        if r >= 0:
                            nc.vector.tensor_tensor(
                                sc2[:, :, r : r + P],
                                sc2[:, :, r : r + P],
                                mask_s[:, None, :].broadcast_to([P, 2, P]),
                                mybir.AluOpType.add,
                            )
                        p2 = pt.tile([P, 2, QT_W], BF16, name="p2")
                        nc.scalar.activation(
                            p2[:, :, r0:QT_W],
                            sc2[:, :, r0:QT_W],
                            AF.Exp,
                            scale=0.125,
                        )
                        for slot in (0, 1):
                            nc.tensor.matmul(
                                ctx[slot][0:65, r0:QT_W],
                                vA[:, kb, slot, :],
                                p2[:, slot, r0:QT_W],
                                start=(kb == 0),
                                stop=(kb == nkb - 1),
                            )
                    # softmax normalization; cT rows 0:64 slot0, 64:128 slot1
                    for slot in (0, 1):
                        # the custom-DVE reciprocal can't read PSUM; stage the
                        # denominator row through SBUF first
                        dsb = sm.tile([1, QT_W], F32, name="dsb")
                        nc.vector.tensor_copy(dsb[:], ctx[slot][64:65, :])
                        lr = sm.tile([1, QT_W], F32, name="lr")
                        nc.vector.reciprocal_approx_fast(lr[:], dsb[:])
                        lb = sm.tile([64, QT_W], F32, name="lb")
                        nc.gpsimd.partition_broadcast(lb[:], lr[0:1, :])
                        nc.vector.tensor_tensor(
                            cT[slot * HD : (slot + 1) * HD, tsl],
                            ctx[slot][0:64, :],
                            lb[:],
                            mybir.AluOpType.mult,
                        )
                outproj(NQT - 1)

    nc.compile()
    return nc


def _host_inputs(x, W_query, W_key, W_value, W_out):
    mask = np.where(
        np.arange(P)[:, None] <= np.arange(P)[None, :], 0.0, NEG
    ).astype(np.float32)
    # host-side transpose: xt[p, c, s] = x[s, c*128 + p]
    xt = np.ascontiguousarray(
        x.astype(BF).T.reshape(KC, P, S).transpose(1, 0, 2)
    )
    in_maps = []
    for core in range(8):
        ha, hb = SLOTS[core]
        sa, sb = SCALES[core]
        ca, cb = slice(ha * HD, (ha + 1) * HD), slice(hb * HD, (hb + 1) * HD)
        # packed per-core projection weights [768, 128] -> [128(p), 6(c), 128]
        def pack(wm):
            sel = np.concatenate([wm[:, ca], wm[:, cb]], axis=1)  # [768, 128]
            return sel.reshape(KC, P, P).transpose(1, 0, 2)  # [p, c, m]

        wq, wk, wv = pack(W_query), pack(W_key), pack(W_value)
        w_all = np.concatenate([wq, wk, wv], axis=2).astype(BF)  # [128, 6, 384]
        wo = np.concatenate([W_out[ca, :] * sa, W_out[cb, :] * sb], axis=0).astype(
            BF
        )
        in_maps.append(
            {
                "xt": xt,
                "w": np.ascontiguousarray(w_all),
                "wo": np.ascontiguousarray(wo),
                "mask": mask,
            }
        )
    return in_maps


def run(x, W_query, W_key, W_value, W_out, b_out, trace=False):
    global _CACHED_NC
    if _CACHED_NC is None:
        _CACHED_NC = build_nc()
    nc = _CACHED_NC
    in_maps = _host_inputs(x, W_query, W_key, W_value, W_out)
    res = run_bass_kernel_spmd(nc, in_maps, core_ids=list(range(8)), trace=trace)
    out = np.zeros((S, D), dtype=np.float32)
    for core in range(8):
        out += res.results[core]["out"]
    out += b_out[None, :].astype(np.float32)
    return out, res


def kernel(x, W_query, W_key, W_value, W_out, b_out):
    x2 = np.asarray(x, dtype=np.float32).reshape(S, D)
    out, _ = run(
        x2,
        np.asarray(W_query, np.float32),
        np.asarray(W_key, np.float32),
        np.asarray(W_value, np.float32),
        np.asarray(W_out, np.float32),
        np.asarray(b_out, np.float32),
    )
    return out.reshape(1, S, D)


# revision 13
# speedup vs baseline: 1.5167x; 1.0237x over previous
"""Causal multi-head attention (B=1, S=4096, D=768, H=12, d_head=64) on 8
Trainium2 NeuronCores.

Sharding: tensor-parallel over heads. 12 heads are mapped onto 16 head-slots
(2 per core); the 4 leftover heads are duplicated onto two slots of the same
core with their W_out rows pre-scaled by 0.5, keeping the SPMD program
uniform across cores. Each core computes Q/K/V projections for its 2 head
slots, causal flash-attention (exp without max-subtraction; softmax
denominator obtained free via an appended ones-column on V), and a partial
row-parallel out-projection. The host sums the 8 partial outputs and adds
b_out (the all-reduce step of the row-parallel out projection).

v2: bf16 data path. The host pre-casts x and all weights to bf16; x is
transposed on the fly by the DMA XBAR (dma transpose), so the PE runs no
transposes at all. Matmuls stream bf16 (1 cycle/row, lower power than
float32r -> less HAM throttling). Scores for the two head slots go to one
2-bank PSUM tile so a single activation instruction exponentiates both
slots. Softmax reciprocal uses the fast approximate DVE op. Per-q-tile
interleaving: project tile t, then attention for tile t, with tile t-1's
out-projection slotted between to hide the softmax-normalize latency.
"""

import sys

sys.path.insert(0, "/opt/trn_rl_repo")

import ml_dtypes
import numpy as np

import concourse.bass as bass
import concourse.tile as tile
from concourse import bacc, mybir
from concourse.bass_utils import run_bass_kernel_spmd

S = 4096
D = 768
HD = 64
P = 128
KC = D // P  # 6 contraction chunks for the projections
QT_W = 512  # query-tile width (psum free dim)
NQT = S // QT_W  # 8 query tiles
NKB = S // P  # 32 key blocks
NEG = -1e30

F32 = mybir.dt.float32
BF16 = mybir.dt.bfloat16
AF = mybir.ActivationFunctionType
BF = ml_dtypes.bfloat16

SLOTS = [(0, 1), (2, 3), (4, 5), (6, 7), (8, 8), (9, 9), (10, 10), (11, 11)]
SCALES = [(1.0, 1.0)] * 4 + [(0.5, 0.5)] * 4

_CACHED_NC = None


def build_nc():
    nc = bacc.Bacc("TRN2", target_bir_lowering=False, debug=False, num_devices=8)

    xt_d = nc.declare_dram_parameter("xt", [P, KC, S], BF16, isOutput=False)
    w_d = nc.declare_dram_parameter("w", [P, KC, 3 * P], BF16, isOutput=False)
    wo_d = nc.declare_dram_parameter("wo", [P, D], BF16, isOutput=False)
    mask_d = nc.declare_dram_parameter("mask", [P, P], F32, isOutput=False)
    out_d = nc.declare_dram_parameter("out", [S, D], F32, isOutput=True)

    with tile.TileContext(nc) as tc:
        with (
            tc.tile_pool(name="const", bufs=1) as const,
            tc.tile_pool(name="big", bufs=1) as big,
        ):
            # ---- constants / staging ----
            warm = const.tile([P, 256], BF16)
            nc.gpsimd.memset(warm[:], 0.0)
            mask_s = const.tile([P, P], F32)
            nc.sync.dma_start(mask_s[:], mask_d[:])
            w = const.tile([P, KC, 3 * P], BF16)
            nc.sync.dma_start(w[:], w_d[:])
            wo = const.tile([P, D], BF16)
            nc.sync.dma_start(wo[:], wo_d[:])

            # qT/kT: packed 2-slot layout straight from the projection PSUM:
            # slot s occupies partitions s*64:(s+1)*64; scores contract K=64.
            xT = big.tile([P, KC, S], BF16)
            qT = big.tile([P, S], BF16)
            kT = big.tile([P, S], BF16)
            # vA: per key block, per slot: [64 v-dims + ones column] so the PV
            # matmul's lhsT [128 keys, 65] yields ctx rows 0:64 and the
            # softmax denominator in row 64.
            vA = big.tile([P, NKB, 2, 65], BF16)
            nc.gpsimd.memset(vA[:, :, :, 64:65], 1.0)
            cT = big.tile([P, S], BF16)
            scratch = const.tile([P, P], F32)

            # xT comes pre-transposed from the host; one DMA per q-tile (all 6
            # chunks) keeps the sync queue's per-DMA issue overhead off the
            # critical path while projections still only wait on their tile.
            for t in range(NQT):
                nc.sync.dma_start(
                    xT[:, :, t * QT_W : (t + 1) * QT_W],
                    xt_d[:, :, t * QT_W : (t + 1) * QT_W],
                )

            with (
                tc.tile_pool(name="pjp", bufs=2, space="PSUM") as pjp,
                tc.tile_pool(name="scp", bufs=2, space="PSUM") as scp,
                tc.tile_pool(name="ctp", bufs=1, space="PSUM") as ctp,
                tc.tile_pool(name="vt", bufs=2) as vtp,
                tc.tile_pool(name="pt", bufs=4) as pt,
                tc.tile_pool(name="sm", bufs=4) as sm,
            ):
                # warm up the PE HAM + preload the Exp table while DMAs run
                for wi in range(48):
                    wps = pjp.tile([P, 256], F32, name="warm_ps", tag="pj")
                    nc.tensor.matmul(
                        wps[:], warm[:, 0:P], warm[:], start=True, stop=True
                    )
                nc.scalar.activation(scratch[:], mask_s[:], AF.Exp, scale=0.125)

                def outproj(t):
                    for st in range(4 * t, 4 * t + 4):
                        o_stage = sm.tile([P, D], F32, name="o_stage", bufs=3)
                        for nch in range(2):
                            po = pjp.tile([P, QT_W], F32, name="po", tag="pj")
                            nc.tensor.matmul(
                                po[:, : D // 2],
                                cT[:, st * P : (st + 1) * P],
                                wo[:, nch * (D // 2) : (nch + 1) * (D // 2)],
                                start=True,
                                stop=True,
                            )
                            nc.vector.tensor_copy(
                                o_stage[:, nch * (D // 2) : (nch + 1) * (D // 2)],
                                po[:, : D // 2],
                            )
                        nc.sync.dma_start(
                            out_d[st * P : (st + 1) * P, :], o_stage[:]
                        )

                def proj(t):
                    tsl = slice(t * QT_W, (t + 1) * QT_W)
                    for pi, dst in ((0, qT), (1, kT)):
                        pj = pjp.tile([P, QT_W], F32, name="pj", tag="pj")
                        for c in range(KC):
                            nc.tensor.matmul(
                                pj[:],
                                w[:, c, pi * P : (pi + 1) * P],
                                xT[:, c, tsl],
                                start=(c == 0),
                                stop=(c == KC - 1),
                            )
                        nc.vector.tensor_copy(dst[:, tsl], pj[:])
                    pj = pjp.tile([P, QT_W], F32, name="pj", tag="pj")
                    for c in range(KC):
                        nc.tensor.matmul(
                            pj[:],
                            w[:, c, 2 * P : 3 * P],
                            xT[:, c, tsl],
                            start=(c == 0),
                            stop=(c == KC - 1),
                        )
                    vt = vtp.tile([P, QT_W], BF16, name="vt")
                    nc.vector.tensor_copy(vt[:], pj[:])
                    # V back to natural [keys, vdims] layout via DMA XBAR.
                    # The XBAR needs a contiguous destination, so land in a
                    # staging tile and split the two slots with DVE copies.
                    for b in range(4):
                        kb = 4 * t + b
                        vN = vtp.tile([P, P], BF16, name="vN", bufs=3)
                        nc.sync.dma_start(
                            vN[:],
                            vt[:, b * P : (b + 1) * P],
                            transpose=True,
                        )
                        for slot in (0, 1):
                            nc.vector.tensor_copy(
                                vA[:, kb, slot, 0:64],
                                vN[:, slot * HD : (slot + 1) * HD],
                            )

                # two-tile projection lookahead: attention(t) then proj(t+2)
                # keeps attention phases back-to-back on the PE queue so the
                # activation engine (the attention-phase pacer) never starves;
                # proj/outproj fill the PE while tile t's softmax-normalize
                # (DVE) releases the ctx accumulators for tile t+1.
                proj(0)
                proj(1)
                for t in range(NQT):
                    tsl = slice(t * QT_W, (t + 1) * QT_W)
                    # ---- attention for q-tile t ----
                    nkb = 4 * (t + 1)
                    ctx = [
                        ctp.tile([P, QT_W], F32, name=f"ctx{s}", tag=f"ctx{s}")
                        for s in (0, 1)
                    ]
                    for kb in range(nkb):
                        r = kb * P - t * QT_W  # diagonal offset
                        r0 = max(0, r)
                        sc2 = scp.tile([P, 2, QT_W], F32, name="sc", tag="sc")
                        for slot in (0, 1):
                            ssl = slice(slot * HD, (slot + 1) * HD)
                            nc.tensor.matmul(
                                sc2[:, slot, r0:QT_W],
                                kT[ssl, kb * P : (kb + 1) * P],
                                qT[ssl, t * QT_W + r0 : (t + 1) * QT_W],
                                start=True,
                                stop=True,
                            )
                        if r >= 0:
                            nc.vector.tensor_tensor(
                                sc2[:, :, r : r + P],
                                sc2[:, :, r : r + P],
                                mask_s[:, None, :].broadcast_to([P, 2, P]),
                                mybir.AluOpType.add,
                            )
                        p2 = pt.tile([P, 2, QT_W], BF16, name="p2")
                        nc.scalar.activation(
                            p2[:, :, r0:QT_W],
                            sc2[:, :, r0:QT_W],
                            AF.Exp,
                            scale=0.125,
                        )
                        for slot in (0, 1):
                            nc.tensor.matmul(
                                ctx[slot][0:65, r0:QT_W],
                                vA[:, kb, slot, :],
                                p2[:, slot, r0:QT_W],
                                start=(kb == 0),
                                stop=(kb == nkb - 1),
                            )
                    # softmax normalization; cT rows 0:64 slot0, 64:128 slot1
                    for slot in (0, 1):
                        # the custom-DVE reciprocal can't read PSUM; stage the
                        # denominator row through SBUF first
                        dsb = sm.tile([1, QT_W], F32, name="dsb")
                        nc.vector.tensor_copy(dsb[:], ctx[slot][64:65, :])
                        lr = sm.tile([1, QT_W], F32, name="lr")
                        nc.vector.reciprocal_approx_fast(lr[:], dsb[:])
                        lb = sm.tile([64, QT_W], F32, name="lb")
                        nc.gpsimd.partition_broadcast(lb[:], lr[0:1, :])
                        nc.vector.tensor_tensor(
                            cT[slot * HD : (slot + 1) * HD, tsl],
                            ctx[slot][0:64, :],
                            lb[:],
                            mybir.AluOpType.mult,
                        )
                    if t + 2 < NQT:
                        proj(t + 2)
                    if t > 0:
                        outproj(t - 1)
                outproj(NQT - 1)

    nc.compile()
    return nc


def _host_inputs(x, W_query, W_key, W_value, W_out):
    mask = np.where(
        np.arange(P)[:, None] <= np.arange(P)[None, :], 0.0, NEG
    ).astype(np.float32)
    # host-side transpose: xt[p, c, s] = x[s, c*128 + p]
    xt = np.ascontiguousarray(
        x.astype(BF).T.reshape(KC, P, S).transpose(1, 0, 2)
    )
    in_maps = []
    for core in range(8):
        ha, hb = SLOTS[core]
        sa, sb = SCALES[core]
        ca, cb = slice(ha * HD, (ha + 1) * HD), slice(hb * HD, (hb + 1) * HD)
        # packed per-core projection weights [768, 128] -> [128(p), 6(c), 128]
        def pack(wm):
            sel = np.concatenate([wm[:, ca], wm[:, cb]], axis=1)  # [768, 128]
            return sel.reshape(KC, P, P).transpose(1, 0, 2)  # [p, c, m]

        wq, wk, wv = pack(W_query), pack(W_key), pack(W_value)
        w_all = np.concatenate([wq, wk, wv], axis=2).astype(BF)  # [128, 6, 384]
        wo = np.concatenate([W_out[ca, :] * sa, W_out[cb, :] * sb], axis=0).astype(
            BF
        )
        in_maps.append(
            {
                "xt": xt,
                "w": np.ascontiguousarray(w_all),
                "wo": np.ascontiguousarray(wo),
                "mask": mask,
            }
        )
    return in_maps


def run(x, W_query, W_key, W_value, W_out, b_out, trace=False):
    global _CACHED_NC
    if _CACHED_NC is None:
        _CACHED_NC = build_nc()
    nc = _CACHED_NC
    in_maps = _host_inputs(x, W_query, W_key, W_value, W_out)
    res = run_bass_kernel_spmd(nc, in_maps, core_ids=list(range(8)), trace=trace)
    out = np.zeros((S, D), dtype=np.float32)
    for core in range(8):
        out += res.results[core]["out"]
    out += b_out[None, :].astype(np.float32)
    return out, res


def kernel(x, W_query, W_key, W_value, W_out, b_out):
    x2 = np.asarray(x, dtype=np.float32).reshape(S, D)
    out, _ = run(
        x2,
        np.asarray(W_query, np.float32),
        np.asarray(W_key, np.float32),
        np.asarray(W_value, np.float32),
        np.asarray(W_out, np.float32),
        np.asarray(b_out, np.float32),
    )
    return out.reshape(1, S, D)
